# revision 37
# baseline (speedup 1.0000x reference)
"""Butterworth IIR (order 4) over [B=128, T=160000, 1] on 8 TRN2 NeuronCores.

Strategy: a stable IIR's impulse response decays geometrically (max pole
radius ~0.668 here), so the filter is numerically exactly (tail < 3e-23)
a 128-tap causal FIR:  y[t] = sum_{k<128} h[k] x[t-k].

Chunking time into 128-sample chunks, with X[c, m] = x[c*128 + m]:
    y[c*128 + j] = sum_m X[c, m] H0[m, j] + sum_m X[c-1, m] H1[m, j]
    H0[m, j] = h[j - m]        (0 <= j - m < 128)
    H1[m, j] = h[j - m + 128]  (0 <= j - m + 128 < 128)

On device this is two accumulating TensorE matmuls per window with the
small fixed H matrices as the stationary operand and a phase-major
(transposed) view of x as the wide moving operand.

v2 layout: everything partition-major in DRAM so DMA transfers are few
and large (640KB+ each, 5KB contiguous per partition line):
  xt [128, 16*1251] fp16  (per-seq blocks of 1251 cols, col 0 zeros)
  yt [128, 16*1250] fp16  (cast to fp16 on-device during PSUM evacuation;
                           host casts back to f32 - rel tolerance is 2e-2)
Matmuls run per-seq as an H0 pass then an H1 pass (weights change twice
per sequence, not per window) accumulating in 3 PSUM banks per seq.

Sharding: pure data-parallel, batch 128 -> 16 sequences per core.
"""

import numpy as np

B_FULL = 128
T_FULL = 160000
N_CORES = 8
SEQ_PER_CORE = B_FULL // N_CORES  # 16
CHUNK = 128
NCHUNK = T_FULL // CHUNK  # 1250
TAPS = 128
NWIN = 512  # matmul moving-operand width (chunks per matmul)
XCOLS = SEQ_PER_CORE * (NCHUNK + 1)  # 20016
YCOLS = SEQ_PER_CORE * NCHUNK  # 20000

_NC_CACHE = {}


def _impulse_response(b, a, n):
    """First n samples of the IIR impulse response, computed in float64
    via the same direct-form II transposed recurrence as the reference."""
    b = np.asarray(b, np.float64)
    a = np.asarray(a, np.float64)
    bn = b / a[0]
    an = a / a[0]
    order = len(a) - 1
    z = np.zeros(order, np.float64)
    h = np.zeros(n, np.float64)
    xt = 1.0
    for t in range(n):
        yt = bn[0] * xt + z[0]
        znew = np.empty_like(z)
        znew[:-1] = z[1:] + xt * bn[1:-1] - yt * an[1:-1]
        znew[-1] = xt * bn[-1] - yt * an[-1]
        z = znew
        h[t] = yt
        xt = 0.0
    return h


def _build_h_matrices(b, a):
    h = _impulse_response(b, a, TAPS)
    m = np.arange(CHUNK)[:, None]
    j = np.arange(CHUNK)[None, :]
    d0 = j - m
    d1 = j - m + CHUNK
    H0 = np.where((d0 >= 0) & (d0 < TAPS), h[np.clip(d0, 0, TAPS - 1)], 0.0)
    H1 = np.where((d1 >= 0) & (d1 < TAPS), h[np.clip(d1, 0, TAPS - 1)], 0.0)
    return np.concatenate([H0, H1], axis=1).astype(np.float16)  # [128, 256]


def _build_nc():
    import concourse.bacc as bacc
    import concourse.mybir as mybir
    from concourse.tile import TileContext

    f32 = mybir.dt.float32
    f16 = mybir.dt.float16
    nc = bacc.Bacc()
    xt = nc.declare_dram_parameter("xt", [CHUNK, XCOLS], f16, isOutput=False)
    hh = nc.declare_dram_parameter("hh", [CHUNK, 2 * CHUNK], f16, isOutput=False)
    yt = nc.declare_dram_parameter("yt", [CHUNK, YCOLS], f16, isOutput=True)

    SPS = NCHUNK + 1  # x cols per seq

    # H, and the head of x, as raw SBUF tensors loaded by DMAs emitted
    # BEFORE the TileContext: these land in the 'main' block and execute
    # right after the framework entry barrier (~6us), about 1us before
    # the tile block's first instruction can issue - the wire (the
    # bottleneck) starts that much earlier. An explicit semaphore carries
    # the dependency into the tile context (its shadow-memory tracker
    # only sees in-context instructions).
    h_tile = nc.alloc_sbuf_tensor("h_sb", [CHUNK, 2 * CHUNK], f16)
    x_tile = nc.alloc_sbuf_tensor("x_sb", [CHUNK, XCOLS], f16)
    head_sem = nc.alloc_semaphore("head_sem")
    nc.sync.dma_start(out=h_tile[:], in_=hh[:]).then_inc(head_sem, 16)
    nc.sync.dma_start(out=x_tile[:, 0:130], in_=xt[:, 0:130]).then_inc(head_sem, 16)
    nc.sync.dma_start(out=x_tile[:, 130:SPS], in_=xt[:, 130:SPS]).then_inc(
        head_sem, 16
    )
    # PE blocks here, in the 'main' block, until the head DMAs land; the
    # tile scheduler's deadlock simulator only simulates the tile block,
    # so the externally-incremented semaphore never confuses it.
    nc.tensor.wait_ge(head_sem, 48)

    with TileContext(nc) as tc:
        with (
            tc.tile_pool(name="const", bufs=1) as cpool,
            tc.tile_pool(name="yout", bufs=1) as ypool,
            tc.tile_pool(name="acc", bufs=8, space="PSUM") as pspool,
        ):
            y_tile = ypool.tile([CHUNK, YCOLS], f16)

            # Pre-warm the scalar engine's activation table (one-time
            # ACT_TABLE_LOAD ~1.3us) so it doesn't stall the first real
            # PSUM evacuation. The value read is discarded.
            warm = cpool.tile([1, 4], f16, name="warm")
            nc.scalar.copy(out=warm[:], in_=h_tile[0:1, 0:4])

            # Remaining input on the single sync HWDGE ring: the ring is
            # FIFO, so queueing every input chunk before any output chunk
            # gives the input stream strict wire priority. Input then
            # finishes ~8us earlier, compute drains right behind it, and
            # the queued output backlog keeps the wire busy to the end
            # with no cast-starvation gaps.
            # 2-seq 640KB chunks (5004B partition lines) measured fastest:
            # bigger 1.28MB chunks fragment to 4096+4096+1816B packets and
            # drop the wire from ~354 to ~322 GB/s.
            in_bounds = [2 * SPS] + [(4 + 2 * k) * SPS for k in range(7)]
            prev = SPS
            for b_ in in_bounds:
                nc.sync.dma_start(out=x_tile[:, prev:b_], in_=xt[:, prev:b_])
                prev = b_

            # Output: 2-seq chunks on the same sync ring, behind all input;
            # the tail chunks shrink so the final drain is short.
            out_bounds = [2 * k * NCHUNK for k in range(1, 8)] + [15 * NCHUNK, YCOLS]

            base_wins = [(0, NWIN), (NWIN, NWIN), (2 * NWIN, NCHUNK - 2 * NWIN)]
            # seq 0 starts with a 128-chunk window: only 130 input columns
            # (33KB) must land before the first matmul issues.
            first_wins = [(0, 128), (128, NWIN - 128), (NWIN, NWIN), (2 * NWIN, NCHUNK - 2 * NWIN)]

            ob = 0
            for s in range(SEQ_PER_CORE):
                wins = first_wins if s == 0 else base_wins
                xb = s * SPS  # col 0 of this seq block = zeros
                yb = s * NCHUNK
                ps = [
                    pspool.tile([CHUNK, NWIN], f32, name="ps")
                    for i in range(len(wins))
                ]
                # H0 pass (weights stay resident across the windows)
                for (w, n), p in zip(wins, ps):
                    nc.tensor.matmul(
                        p[:, :n],
                        h_tile[:, 0:CHUNK],
                        x_tile[:, xb + 1 + w : xb + 1 + w + n],
                        start=True,
                        stop=False,
                    )
                # H1 pass
                for (w, n), p in zip(wins, ps):
                    nc.tensor.matmul(
                        p[:, :n],
                        h_tile[:, CHUNK : 2 * CHUNK],
                        x_tile[:, xb + w : xb + w + n],
                        start=False,
                        stop=True,
                    )
                # evacuate PSUM -> SBUF with f32->fp16 cast. PSUM f32 reads
                # run at ~2 cycles/col, so one engine alone (~28us) would
                # gate the kernel: within each seq DVE takes all windows but
                # one; scalar takes a 512-col window, so the seq's banks
                # recycle concurrently. (A single per-seq cast is not
                # possible: PSUM access patterns cannot cross 2KB banks.)
                for (w, n), p in zip(wins, ps):
                    if w == NWIN:
                        nc.scalar.copy(
                            out=y_tile[:, yb + w : yb + w + n], in_=p[:, :n]
                        )
                    else:
                        nc.vector.tensor_copy(
                            out=y_tile[:, yb + w : yb + w + n], in_=p[:, :n]
                        )
                ready = (s + 1) * NCHUNK  # y cols evacuated so far
                while ob < len(out_bounds) and out_bounds[ob] <= ready:
                    lo = out_bounds[ob - 1] if ob else 0
                    hi = out_bounds[ob]
                    nc.sync.dma_start(out=yt[:, lo:hi], in_=y_tile[:, lo:hi])
                    ob += 1
    nc.compile()
    return nc


def _run_on_device(in_maps, trace=False):
    from concourse.bass_utils import run_bass_kernel_spmd

    if "nc" not in _NC_CACHE:
        _NC_CACHE["nc"] = _build_nc()
    return run_bass_kernel_spmd(
        _NC_CACHE["nc"], in_maps, core_ids=list(range(N_CORES)), trace=trace
    )


def _prepare_in_maps(x, b, a):
    hh = _build_h_matrices(b, a)
    xs = np.asarray(x, np.float16).reshape(B_FULL, NCHUNK, CHUNK)
    in_maps = []
    for c in range(N_CORES):
        xc = xs[c * SEQ_PER_CORE : (c + 1) * SEQ_PER_CORE]  # [16, 1250, 128]
        xt = np.zeros((SEQ_PER_CORE, NCHUNK + 1, CHUNK), np.float16)
        xt[:, 1:, :] = xc
        # -> [128, 16*(1250+1)] phase-major, partition-major in DRAM
        xt = np.ascontiguousarray(
            xt.transpose(2, 0, 1).reshape(CHUNK, XCOLS)
        )
        in_maps.append({"xt": xt, "hh": hh})
    return in_maps


def _assemble_output(results):
    out = np.empty((B_FULL, T_FULL, 1), np.float32)
    for c in range(N_CORES):
        ytc = np.asarray(results[c]["yt"])  # [128, 16*1250] fp16 phase-major
        yc = ytc.reshape(CHUNK, SEQ_PER_CORE, NCHUNK).transpose(1, 2, 0)
        out[c * SEQ_PER_CORE : (c + 1) * SEQ_PER_CORE, :, 0] = yc.reshape(
            SEQ_PER_CORE, T_FULL
        )
    return out


def kernel(x, b, a):
    in_maps = _prepare_in_maps(x, b, a)
    res = _run_on_device(in_maps, trace=False)
    return _assemble_output(res.results)


def kernel_traced(x, b, a):
    """Same as kernel() but with neuron profiling; returns (output, exec_time_ns)."""
    in_maps = _prepare_in_maps(x, b, a)
    try:
        res = _run_on_device(in_maps, trace=True)
    except ModuleNotFoundError:
        res = _run_on_device(in_maps, trace=False)
    return _assemble_output(res.results), res.exec_time_ns


# revision 38
# speedup vs baseline: 1.0493x; 1.0493x over previous
"""Butterworth IIR (order 4) over [B=128, T=160000, 1] on 8 TRN2 NeuronCores.

Strategy: a stable IIR's impulse response decays geometrically (max pole
radius ~0.668 here), so the filter is numerically exactly (tail < 3e-23)
a 128-tap causal FIR:  y[t] = sum_{k<128} h[k] x[t-k].

Chunking time into 128-sample chunks, with X[c, m] = x[c*128 + m]:
    y[c*128 + j] = sum_m X[c, m] H0[m, j] + sum_m X[c-1, m] H1[m, j]
    H0[m, j] = h[j - m]        (0 <= j - m < 128)
    H1[m, j] = h[j - m + 128]  (0 <= j - m + 128 < 128)

On device this is two accumulating TensorE matmuls per window with the
small fixed H matrices as the stationary operand and a phase-major
(transposed) view of x as the wide moving operand.

v2 layout: everything partition-major in DRAM so DMA transfers are few
and large (640KB+ each, 5KB contiguous per partition line):
  xt [128, 16*1251] fp16  (per-seq blocks of 1251 cols, col 0 zeros)
  yt [128, 16*1250] fp16  (cast to fp16 on-device during PSUM evacuation;
                           host casts back to f32 - rel tolerance is 2e-2)
Matmuls run per-seq as an H0 pass then an H1 pass (weights change twice
per sequence, not per window) accumulating in 3 PSUM banks per seq.

Sharding: pure data-parallel, batch 128 -> 16 sequences per core.
"""

import numpy as np

B_FULL = 128
T_FULL = 160000
N_CORES = 8
SEQ_PER_CORE = B_FULL // N_CORES  # 16
CHUNK = 128
NCHUNK = T_FULL // CHUNK  # 1250
TAPS = 128
NWIN = 512  # matmul moving-operand width (chunks per matmul)
XCOLS = SEQ_PER_CORE * (NCHUNK + 1)  # 20016
YCOLS = SEQ_PER_CORE * NCHUNK  # 20000

_NC_CACHE = {}


def _impulse_response(b, a, n):
    """First n samples of the IIR impulse response, computed in float64
    via the same direct-form II transposed recurrence as the reference."""
    b = np.asarray(b, np.float64)
    a = np.asarray(a, np.float64)
    bn = b / a[0]
    an = a / a[0]
    order = len(a) - 1
    z = np.zeros(order, np.float64)
    h = np.zeros(n, np.float64)
    xt = 1.0
    for t in range(n):
        yt = bn[0] * xt + z[0]
        znew = np.empty_like(z)
        znew[:-1] = z[1:] + xt * bn[1:-1] - yt * an[1:-1]
        znew[-1] = xt * bn[-1] - yt * an[-1]
        z = znew
        h[t] = yt
        xt = 0.0
    return h


def _build_h_matrices(b, a):
    h = _impulse_response(b, a, TAPS)
    m = np.arange(CHUNK)[:, None]
    j = np.arange(CHUNK)[None, :]
    d0 = j - m
    d1 = j - m + CHUNK
    H0 = np.where((d0 >= 0) & (d0 < TAPS), h[np.clip(d0, 0, TAPS - 1)], 0.0)
    H1 = np.where((d1 >= 0) & (d1 < TAPS), h[np.clip(d1, 0, TAPS - 1)], 0.0)
    return np.concatenate([H0, H1], axis=1).astype(np.float16)  # [128, 256]


def _build_nc():
    import concourse.bacc as bacc
    import concourse.mybir as mybir
    from concourse.tile import TileContext

    f32 = mybir.dt.float32
    f16 = mybir.dt.float16
    nc = bacc.Bacc()
    xt = nc.declare_dram_parameter("xt", [CHUNK, XCOLS], f16, isOutput=False)
    hh = nc.declare_dram_parameter("hh", [CHUNK, 2 * CHUNK], f16, isOutput=False)
    yt = nc.declare_dram_parameter("yt", [CHUNK, YCOLS], f16, isOutput=True)

    SPS = NCHUNK + 1  # x cols per seq

    with TileContext(nc) as tc:
        with (
            tc.tile_pool(name="const", bufs=1) as cpool,
            tc.tile_pool(name="xin", bufs=1) as xpool,
            tc.tile_pool(name="yout", bufs=1) as ypool,
            tc.tile_pool(name="acc", bufs=8, space="PSUM") as pspool,
        ):
            h_tile = cpool.tile([CHUNK, 2 * CHUNK], f16)
            # H first on the scalar HWDGE ring (idle at startup, so it
            # issues in parallel with the input stream on sync): it gates
            # the first matmul; HWDGE first-byte (~0.6us) beats SWDGE.
            nc.scalar.dma_start(out=h_tile[:], in_=hh[:])

            x_tile = xpool.tile([CHUNK, XCOLS], f16)
            y_tile = ypool.tile([CHUNK, YCOLS], f16)

            # ALL bulk data DMAs go on the single sync HWDGE ring: the
            # ring is FIFO, so queueing every input chunk before any
            # output chunk gives the input stream strict wire priority.
            # Input then finishes ~8us earlier, compute drains right
            # behind it, and the queued output backlog keeps the wire
            # busy to the end with no cast-starvation gaps.
            # The 33KB head chunk rides the scalar ring (with H) so it
            # issues in parallel with sync's first big chunk - the first
            # matmul's operands land ~1us sooner.
            nc.scalar.dma_start(out=x_tile[:, 0:130], in_=xt[:, 0:130])
            # Pre-warm the scalar engine's activation table (one-time
            # ACT_TABLE_LOAD ~1.3us) after its DMA issues so it doesn't
            # stall the first real PSUM evacuation.
            warm = cpool.tile([1, 4], f16, name="warm")
            nc.scalar.copy(out=warm[:], in_=h_tile[0:1, 0:4])
            # 2-seq 640KB chunks (5004B partition lines) measured fastest:
            # bigger 1.28MB chunks fragment to 4096+4096+1816B packets and
            # drop the wire from ~354 to ~322 GB/s.
            in_bounds = [SPS, 2 * SPS] + [(4 + 2 * k) * SPS for k in range(7)]
            prev = 130
            for b_ in in_bounds:
                nc.sync.dma_start(out=x_tile[:, prev:b_], in_=xt[:, prev:b_])
                prev = b_

            # Output: 2-seq chunks on the same sync ring, behind all input;
            # the tail chunks shrink so the final drain is short.
            out_bounds = [2 * k * NCHUNK for k in range(1, 8)] + [15 * NCHUNK, YCOLS]

            base_wins = [(0, NWIN), (NWIN, NWIN), (2 * NWIN, NCHUNK - 2 * NWIN)]
            # seq 0 starts with a 128-chunk window: only 130 input columns
            # (33KB) must land before the first matmul issues.
            first_wins = [(0, 128), (128, NWIN - 128), (NWIN, NWIN), (2 * NWIN, NCHUNK - 2 * NWIN)]

            ob = 0
            for s in range(SEQ_PER_CORE):
                wins = first_wins if s == 0 else base_wins
                xb = s * SPS  # col 0 of this seq block = zeros
                yb = s * NCHUNK
                ps = [
                    pspool.tile([CHUNK, NWIN], f32, name="ps")
                    for i in range(len(wins))
                ]
                # H0 pass (weights stay resident across the windows)
                for (w, n), p in zip(wins, ps):
                    nc.tensor.matmul(
                        p[:, :n],
                        h_tile[:, 0:CHUNK],
                        x_tile[:, xb + 1 + w : xb + 1 + w + n],
                        start=True,
                        stop=False,
                    )
                # H1 pass
                for (w, n), p in zip(wins, ps):
                    nc.tensor.matmul(
                        p[:, :n],
                        h_tile[:, CHUNK : 2 * CHUNK],
                        x_tile[:, xb + w : xb + w + n],
                        start=False,
                        stop=True,
                    )
                # evacuate PSUM -> SBUF with f32->fp16 cast. PSUM f32 reads
                # run at ~2 cycles/col, so one engine alone (~28us) would
                # gate the kernel: within each seq DVE takes all windows but
                # one; scalar takes a 512-col window, so the seq's banks
                # recycle concurrently. (A single per-seq cast is not
                # possible: PSUM access patterns cannot cross 2KB banks.)
                for (w, n), p in zip(wins, ps):
                    if w == NWIN:
                        nc.scalar.copy(
                            out=y_tile[:, yb + w : yb + w + n], in_=p[:, :n]
                        )
                    else:
                        nc.vector.tensor_copy(
                            out=y_tile[:, yb + w : yb + w + n], in_=p[:, :n]
                        )
                ready = (s + 1) * NCHUNK  # y cols evacuated so far
                while ob < len(out_bounds) and out_bounds[ob] <= ready:
                    lo = out_bounds[ob - 1] if ob else 0
                    hi = out_bounds[ob]
                    nc.sync.dma_start(out=yt[:, lo:hi], in_=y_tile[:, lo:hi])
                    ob += 1
    nc.compile()
    return nc


def _run_on_device(in_maps, trace=False):
    from concourse.bass_utils import run_bass_kernel_spmd

    if "nc" not in _NC_CACHE:
        _NC_CACHE["nc"] = _build_nc()
    return run_bass_kernel_spmd(
        _NC_CACHE["nc"], in_maps, core_ids=list(range(N_CORES)), trace=trace
    )


def _prepare_in_maps(x, b, a):
    hh = _build_h_matrices(b, a)
    xs = np.asarray(x, np.float16).reshape(B_FULL, NCHUNK, CHUNK)
    in_maps = []
    for c in range(N_CORES):
        xc = xs[c * SEQ_PER_CORE : (c + 1) * SEQ_PER_CORE]  # [16, 1250, 128]
        xt = np.zeros((SEQ_PER_CORE, NCHUNK + 1, CHUNK), np.float16)
        xt[:, 1:, :] = xc
        # -> [128, 16*(1250+1)] phase-major, partition-major in DRAM
        xt = np.ascontiguousarray(
            xt.transpose(2, 0, 1).reshape(CHUNK, XCOLS)
        )
        in_maps.append({"xt": xt, "hh": hh})
    return in_maps


def _assemble_output(results):
    out = np.empty((B_FULL, T_FULL, 1), np.float32)
    for c in range(N_CORES):
        ytc = np.asarray(results[c]["yt"])  # [128, 16*1250] fp16 phase-major
        yc = ytc.reshape(CHUNK, SEQ_PER_CORE, NCHUNK).transpose(1, 2, 0)
        out[c * SEQ_PER_CORE : (c + 1) * SEQ_PER_CORE, :, 0] = yc.reshape(
            SEQ_PER_CORE, T_FULL
        )
    return out


def kernel(x, b, a):
    in_maps = _prepare_in_maps(x, b, a)
    res = _run_on_device(in_maps, trace=False)
    return _assemble_output(res.results)


def kernel_traced(x, b, a):
    """Same as kernel() but with neuron profiling; returns (output, exec_time_ns)."""
    in_maps = _prepare_in_maps(x, b, a)
    try:
        res = _run_on_device(in_maps, trace=True)
    except ModuleNotFoundError:
        res = _run_on_device(in_maps, trace=False)
    return _assemble_output(res.results), res.exec_time_ns


# revision 39
# speedup vs baseline: 1.1552x; 1.1010x over previous
"""Butterworth IIR (order 4) over [B=128, T=160000, 1] on 8 TRN2 NeuronCores.

Strategy: a stable IIR's impulse response decays geometrically (max pole
radius ~0.668 here), so the filter is numerically exactly (tail < 3e-23)
a 128-tap causal FIR:  y[t] = sum_{k<128} h[k] x[t-k].

Chunking time into 128-sample chunks, with X[c, m] = x[c*128 + m]:
    y[c*128 + j] = sum_m X[c, m] H0[m, j] + sum_m X[c-1, m] H1[m, j]
    H0[m, j] = h[j - m]        (0 <= j - m < 128)
    H1[m, j] = h[j - m + 128]  (0 <= j - m + 128 < 128)

On device this is two accumulating TensorE matmuls per window with the
small fixed H matrices as the stationary operand and a phase-major
(transposed) view of x as the wide moving operand.

Layout: everything partition-major in DRAM so DMA transfers are few and
large (640KB bulk chunks, ~5KB contiguous per partition line):
  xt [128, 16*1251] fp16  (per-seq blocks of 1251 cols, col 0 zeros)
  yt [128, 16*1250] fp16  (cast to fp16 on-device during PSUM evacuation;
                           host casts back to f32 - rel tolerance is 2e-2)

The kernel is wire-bound (10.24MB at the ~358 GB/s per-core HBM limit),
so everything is organized around keeping the DMA wire at 100% duty:
all bulk transfers ride ONE HWDGE ring (sync) with every input chunk
queued before any output chunk (ring FIFO = strict input priority;
compute drains behind the input stream and the queued output backlog
covers the wire to the end). Matmuls run per-seq as an H0 pass then an
H1 pass (weights change twice per sequence, not per window) into
per-window single-bank PSUM tiles; evacuation is split between the DVE
and scalar engines (PSUM f32 reads cost ~2 cycles/col on either, and
PSUM access patterns must not cross 2KB banks).

Sharding: pure data-parallel, batch 128 -> 16 sequences per core.
"""

import numpy as np

B_FULL = 128
T_FULL = 160000
N_CORES = 8
SEQ_PER_CORE = B_FULL // N_CORES  # 16
CHUNK = 128
NCHUNK = T_FULL // CHUNK  # 1250
TAPS = 128
NWIN = 512  # matmul moving-operand width (chunks per matmul)
XCOLS = SEQ_PER_CORE * (NCHUNK + 1)  # 20016
YCOLS = SEQ_PER_CORE * NCHUNK  # 20000

_NC_CACHE = {}


def _impulse_response(b, a, n):
    """First n samples of the IIR impulse response, computed in float64
    via the same direct-form II transposed recurrence as the reference."""
    b = np.asarray(b, np.float64)
    a = np.asarray(a, np.float64)
    bn = b / a[0]
    an = a / a[0]
    order = len(a) - 1
    z = np.zeros(order, np.float64)
    h = np.zeros(n, np.float64)
    xt = 1.0
    for t in range(n):
        yt = bn[0] * xt + z[0]
        znew = np.empty_like(z)
        znew[:-1] = z[1:] + xt * bn[1:-1] - yt * an[1:-1]
        znew[-1] = xt * bn[-1] - yt * an[-1]
        z = znew
        h[t] = yt
        xt = 0.0
    return h


def _build_h_matrices(b, a):
    h = _impulse_response(b, a, TAPS)
    m = np.arange(CHUNK)[:, None]
    j = np.arange(CHUNK)[None, :]
    d0 = j - m
    d1 = j - m + CHUNK
    H0 = np.where((d0 >= 0) & (d0 < TAPS), h[np.clip(d0, 0, TAPS - 1)], 0.0)
    H1 = np.where((d1 >= 0) & (d1 < TAPS), h[np.clip(d1, 0, TAPS - 1)], 0.0)
    return np.concatenate([H0, H1], axis=1).astype(np.float16)  # [128, 256]


def _build_nc():
    import concourse.bacc as bacc
    import concourse.mybir as mybir
    from concourse.tile import TileContext

    f32 = mybir.dt.float32
    f16 = mybir.dt.float16
    nc = bacc.Bacc()
    xt = nc.declare_dram_parameter("xt", [CHUNK, XCOLS], f16, isOutput=False)
    hh = nc.declare_dram_parameter("hh", [CHUNK, 2 * CHUNK], f16, isOutput=False)
    yt = nc.declare_dram_parameter("yt", [CHUNK, YCOLS], f16, isOutput=True)

    SPS = NCHUNK + 1  # x cols per seq

    with TileContext(nc) as tc:
        with (
            tc.tile_pool(name="const", bufs=1) as cpool,
            tc.tile_pool(name="xin", bufs=1) as xpool,
            tc.tile_pool(name="yout", bufs=1) as ypool,
            tc.tile_pool(name="acc", bufs=8, space="PSUM") as pspool,
        ):
            h_tile = cpool.tile([CHUNK, 2 * CHUNK], f16)
            # H first on the scalar HWDGE ring (idle at startup, so it
            # issues in parallel with the input stream on sync): it gates
            # the first matmul; HWDGE first-byte (~0.6us) beats SWDGE.
            nc.scalar.dma_start(out=h_tile[:], in_=hh[:])

            x_tile = xpool.tile([CHUNK, XCOLS], f16)
            y_tile = ypool.tile([CHUNK, YCOLS], f16)

            # ALL bulk data DMAs go on the single sync HWDGE ring: the
            # ring is FIFO, so queueing every input chunk before any
            # output chunk gives the input stream strict wire priority.
            # Input then finishes ~8us earlier, compute drains right
            # behind it, and the queued output backlog keeps the wire
            # busy to the end with no cast-starvation gaps.
            # The 33KB head chunk rides the scalar ring (with H) so it
            # issues in parallel with sync's first big chunk - the first
            # matmul's operands land ~1us sooner.
            nc.scalar.dma_start(out=x_tile[:, 0:130], in_=xt[:, 0:130])
            # Pre-warm the scalar engine's activation table (one-time
            # ACT_TABLE_LOAD ~1.3us) after its DMA issues so it doesn't
            # stall the first real PSUM evacuation.
            warm = cpool.tile([1, 4], f16, name="warm")
            nc.scalar.copy(out=warm[:], in_=h_tile[0:1, 0:4])
            # 2-seq 640KB chunks (5004B partition lines) measured fastest:
            # bigger 1.28MB chunks fragment to 4096+4096+1816B packets and
            # drop the wire from ~354 to ~322 GB/s.
            in_bounds = [SPS, 2 * SPS] + [(4 + 2 * k) * SPS for k in range(7)]
            prev = 130
            for b_ in in_bounds:
                nc.sync.dma_start(out=x_tile[:, prev:b_], in_=xt[:, prev:b_])
                prev = b_

            # Output: 2-seq chunks on the same sync ring, behind all input;
            # the tail chunks shrink so the final drain is short.
            out_bounds = [2 * k * NCHUNK for k in range(1, 8)] + [15 * NCHUNK, YCOLS]

            base_wins = [(0, NWIN), (NWIN, NWIN), (2 * NWIN, NCHUNK - 2 * NWIN)]
            # seq 0 starts with a 128-chunk window: only 130 input columns
            # (33KB) must land before the first matmul issues.
            first_wins = [(0, 128), (128, NWIN - 128), (NWIN, NWIN), (2 * NWIN, NCHUNK - 2 * NWIN)]

            ob = 0
            for s in range(SEQ_PER_CORE):
                wins = first_wins if s == 0 else base_wins
                xb = s * SPS  # col 0 of this seq block = zeros
                yb = s * NCHUNK
                ps = [
                    pspool.tile([CHUNK, NWIN], f32, name="ps")
                    for i in range(len(wins))
                ]
                # H0 pass (weights stay resident across the windows)
                for (w, n), p in zip(wins, ps):
                    nc.tensor.matmul(
                        p[:, :n],
                        h_tile[:, 0:CHUNK],
                        x_tile[:, xb + 1 + w : xb + 1 + w + n],
                        start=True,
                        stop=False,
                    )
                # H1 pass
                for (w, n), p in zip(wins, ps):
                    nc.tensor.matmul(
                        p[:, :n],
                        h_tile[:, CHUNK : 2 * CHUNK],
                        x_tile[:, xb + w : xb + w + n],
                        start=False,
                        stop=True,
                    )
                # evacuate PSUM -> SBUF with f32->fp16 cast. PSUM f32 reads
                # run at ~2 cycles/col, so one engine alone (~28us) would
                # gate the kernel: within each seq DVE takes all windows but
                # one; scalar takes a 512-col window, so the seq's banks
                # recycle concurrently. (A single per-seq cast is not
                # possible: PSUM access patterns cannot cross 2KB banks.)
                for (w, n), p in zip(wins, ps):
                    if w == NWIN:
                        nc.scalar.copy(
                            out=y_tile[:, yb + w : yb + w + n], in_=p[:, :n]
                        )
                    else:
                        nc.vector.tensor_copy(
                            out=y_tile[:, yb + w : yb + w + n], in_=p[:, :n]
                        )
                ready = (s + 1) * NCHUNK  # y cols evacuated so far
                while ob < len(out_bounds) and out_bounds[ob] <= ready:
                    lo = out_bounds[ob - 1] if ob else 0
                    hi = out_bounds[ob]
                    nc.sync.dma_start(out=yt[:, lo:hi], in_=y_tile[:, lo:hi])
                    ob += 1
    nc.compile()
    return nc


def _run_on_device(in_maps, trace=False):
    from concourse.bass_utils import run_bass_kernel_spmd

    if "nc" not in _NC_CACHE:
        _NC_CACHE["nc"] = _build_nc()
    return run_bass_kernel_spmd(
        _NC_CACHE["nc"], in_maps, core_ids=list(range(N_CORES)), trace=trace
    )


def _prepare_in_maps(x, b, a):
    hh = _build_h_matrices(b, a)
    xs = np.asarray(x, np.float16).reshape(B_FULL, NCHUNK, CHUNK)
    in_maps = []
    for c in range(N_CORES):
        xc = xs[c * SEQ_PER_CORE : (c + 1) * SEQ_PER_CORE]  # [16, 1250, 128]
        xt = np.zeros((SEQ_PER_CORE, NCHUNK + 1, CHUNK), np.float16)
        xt[:, 1:, :] = xc
        # -> [128, 16*(1250+1)] phase-major, partition-major in DRAM
        xt = np.ascontiguousarray(
            xt.transpose(2, 0, 1).reshape(CHUNK, XCOLS)
        )
        in_maps.append({"xt": xt, "hh": hh})
    return in_maps


def _assemble_output(results):
    out = np.empty((B_FULL, T_FULL, 1), np.float32)
    for c in range(N_CORES):
        ytc = np.asarray(results[c]["yt"])  # [128, 16*1250] fp16 phase-major
        yc = ytc.reshape(CHUNK, SEQ_PER_CORE, NCHUNK).transpose(1, 2, 0)
        out[c * SEQ_PER_CORE : (c + 1) * SEQ_PER_CORE, :, 0] = yc.reshape(
            SEQ_PER_CORE, T_FULL
        )
    return out


def kernel(x, b, a):
    in_maps = _prepare_in_maps(x, b, a)
    res = _run_on_device(in_maps, trace=False)
    return _assemble_output(res.results)


def kernel_traced(x, b, a):
    """Same as kernel() but with neuron profiling; returns (output, exec_time_ns)."""
    in_maps = _prepare_in_maps(x, b, a)
    try:
        res = _run_on_device(in_maps, trace=True)
    except ModuleNotFoundError:
        res = _run_on_device(in_maps, trace=False)
    return _assemble_output(res.results), res.exec_time_ns
